# revision 44
# baseline (speedup 1.0000x reference)
"""Trainium2 Bass kernel for sliding-window unfold (im2col).

reference:  out = x[:, idx, :]  with idx[w, f] = w + f
  x:   [128, 4096, 4]  f32
  out: [128, 4065, 32, 4]  f32

Key structural fact: out[b, w] (= 32*4 = 128 floats = 512 B) is the
contiguous slice x[b].flat[4w : 4w + 128].  The whole problem is a
sliding-window byte replication; HBM write bandwidth is the roofline.

Measured on TRN2: a SWDGE dma_start spanning EXACTLY 128 partitions is
sprayed across all 16 SDMA engines (~340-370 GB/s); any other partition
count executes on a single engine (~20 GB/s).  So every bulk transfer
here spans 128 partitions.

Strategy (pure data parallel, batch 128 -> 16 per core on 8 cores),
per batch b on each core:
  1. one DMA loads a replicated tile X[128, 248]: partition p holds
     x[b].flat[124p : 124p+248] - everything windows 31p..31p+30 touch.
  2. one DVE copy expands X -> Y[128, 3968] with an overlapping-stride
     read AP: Y[p, 128j+i] = X[p, 4j+i] -> partition p holds windows
     31p..31p+30 materialized contiguously (15.5 KB).
  3. one 128-partition DMA stores Y to out[b] windows 0..3967
     (contiguous 15.5 KB runs per partition - full-rate descriptors).
  4. the 97 ragged tail windows (3968..4064) ride a second 128-partition
     load+store pair covering windows 3937..4064 (one 512 B window per
     partition); the first 31 rows rewrite bulk output with identical
     bytes, keeping both transfers on the fast path.

Loads ride the HWDGE rings (nc.sync for X, nc.scalar for the tail tile)
so they interleave with SWDGE store packets instead of queueing behind
them in the SDMA engine FIFOs; expands alternate DVE/ACT.
"""

import numpy as np

from concourse import bacc, mybir, tile
from concourse.bass_utils import run_bass_kernel_spmd

N_CORES = 8
B_FULL = 128
B = B_FULL // N_CORES  # 16 batches per core
S = 4096
C = 4
F = 32
W = S - F + 1    # 4065
FL = F * C       # 128 floats per window
XB = S * C       # 16384 floats per batch of x
OB = W * FL      # 520320 floats per batch of out
WPP = 31         # windows per partition in the bulk store
NBULK = 128 * WPP          # 3968 bulk windows per batch
NTAIL = W - NBULK          # 97 tail windows
YROW = WPP * FL            # 3968 floats per partition row
XROW = (WPP - 1) * C + FL  # 248 floats of x per partition

_cache = {}


def build_nc():
    nc = bacc.Bacc("TRN2", target_bir_lowering=False)
    x = nc.dram_tensor("x", [B, S, C], mybir.dt.float32, kind="ExternalInput")
    out = nc.dram_tensor("out", [B, W, F, C], mybir.dt.float32, kind="ExternalOutput")

    with tile.TileContext(nc) as tc:
        with (
            tc.tile_pool(name="xp", bufs=8) as xp,
            tc.tile_pool(name="yp", bufs=10) as yp,
            tc.tile_pool(name="tp", bufs=4) as tp,
        ):
            def load(b):
                X = xp.tile([128, XROW], mybir.dt.float32)
                src = x[:].copy()
                src.ap = mybir.VecI64Pair([[WPP * C, 128], [1, XROW]])
                src.offset = b * XB
                nc.sync.dma_start(out=X[:, :], in_=src)
                return X

            def tail(b):
                # tail: windows 3937..4064 as a 128-partition (fast-path)
                # load+store pair; the first 31 windows duplicate bulk
                # partition 127's output with identical bytes.
                TB = tp.tile([128, FL], mybir.dt.float32)
                srcT = x[:].copy()
                srcT.ap = mybir.VecI64Pair([[C, 128], [1, FL]])
                srcT.offset = b * XB + (NBULK - 31) * C
                nc.scalar.dma_start(out=TB[:, :], in_=srcT)
                dstT = out[:].copy()
                dstT.ap = mybir.VecI64Pair([[FL, 128], [1, FL]])
                dstT.offset = b * OB + (NBULK - 31) * FL
                nc.gpsimd.dma_start(out=dstT, in_=TB[:, :])

            for b in range(B):
                X = load(b)
                Y = yp.tile([128, YROW], mybir.dt.float32)
                src2 = X[:].copy()
                src2.ap = mybir.VecI64Pair([[XROW, 128], [C, WPP], [1, FL]])
                src2.offset = 0
                dst2 = Y[:].copy()
                dst2.ap = mybir.VecI64Pair([[YROW, 128], [FL, WPP], [1, FL]])
                dst2.offset = 0
                if b % 2 == 0:
                    nc.vector.tensor_copy(out=dst2, in_=src2)
                else:
                    nc.scalar.copy(out=dst2, in_=src2)

                dst3 = out[:].copy()
                dst3.ap = mybir.VecI64Pair([[YROW, 128], [1, YROW]])
                dst3.offset = b * OB
                nc.gpsimd.dma_start(out=dst3, in_=Y[:, :])
                tail(b)

    nc.finalize()
    return nc


def run_sharded(x: np.ndarray, trace: bool = False):
    """Shard batch across 8 cores, run, gather. Returns (out, raw results)."""
    if "nc" not in _cache:
        _cache["nc"] = build_nc()
    nc = _cache["nc"]

    x = np.ascontiguousarray(x, dtype=np.float32)
    in_maps = [{"x": x[i * B : (i + 1) * B]} for i in range(N_CORES)]
    res = run_bass_kernel_spmd(nc, in_maps, list(range(N_CORES)), trace=trace)
    out = np.concatenate([res.results[i]["out"] for i in range(N_CORES)], axis=0)
    return out, res


def kernel(x: np.ndarray) -> np.ndarray:
    out, _ = run_sharded(x, trace=False)
    return out
